# revision 78
# baseline (speedup 1.0000x reference)
"""Causal MHA with RoPE on 8 Trainium2 NeuronCores (all-bf16, flipped AV).

Sharding: core c -> batch b=c//2, head-group g=c%2 (8 heads of 16).
Each core: Q/K/V projections for its 512 head-dims over the full sequence,
causal attention for its 8 heads, partial output projection (its 512 rows
of wo). Host sums the two partial (bf16) outputs per batch. No collectives.

Design (timeline-sim 224us/core, down from 239us; hw rel err 3.8e-3):
- All matmul operands bf16 (1.0 cycles/row); ~493K total PE rows at the
  2.4GHz full clock is the ~205us floor this schedule approaches.
- PE p-state warm-up: a few junk matmuls on memset tiles set the busy-start
  early, and four "gate" matmuls parked in the 4-deep wait queue behind a
  DVE delay chain hold back the real matmuls' pipeline entry until the 3us
  ramp window has passed -- every real matmul is costed at full clock.
- Warmup DMA: pass-0 x loads straight (split) and is transposed on PE via
  identity matmuls (an xbar DMA-transpose barriers the whole DMA stream:
  later DMAs wait its completion semaphore); wk is split in halves; order
  x, wk, trig0, psw, wq, wv, wo. First rope matmul ~7.8us vs 12.2us.
- x^T (passes 1-3) and ctx^T (blocks 0-2) still use the xbar DMA-transpose
  (14ns per 16x128 tile) -- prefetched, off the critical path.
- AV matmul flipped to out[queries, 65]: PE cost is charged per output
  free-element, so AV drops ~139K -> ~71K rows; the V ones-column makes
  row 64 the softmax denominator, landing per-query denominators on PSUM
  partitions so normalization is reciprocal [128,1] + one per-partition
  tensor_scalar multiply.
- RoPE: dst = acc*cos - P_swap@(acc*sin); the 32-row block swap runs on PE
  as a permutation matmul into PSUM. Pass-0 emits all staged chains with
  swaps one step behind, so nothing waits on the DVE products.
- Scores at tile-minimum rows: the diagonal chunks' second key tile starts
  its queries 128 later; the unwritten PSUM strip is exp'd (stale scores
  stay small) and zeroed by the causal mask fill.
- Causal masking post-exp via gpsimd affine_select (Pool); the final head's
  masks run as DVE 0/1-mask multiplies so the drain AV chains aren't gated
  on the Pool launch latency.
- Last pass drains through split output projections: ctx^T chunks c=0/1
  (heads 0-3) transpose on PE and project mid-pass into bf16 partials
  staged in a dead x^T buffer; after the final exps only the c=2/3 halves
  plus DVE partial-adds remain, and ACT (idle then, PSUM-capable) does the
  drain transpose copies.
"""

import math

import numpy as np

import concourse.bass as bass
import concourse.mybir as mybir
import concourse.tile as tile
from concourse import bacc
from concourse.bass_utils import run_bass_kernel_spmd
from concourse.masks import make_identity

F32 = mybir.dt.float32
BF16 = mybir.dt.bfloat16

B, S, D, H = 4, 2048, 1024, 16
HD = D // H          # 64
THETA = 10000.0
DH = D // 2          # 512 per-core head dims (8 heads)
NP = 4               # head pairs per core
NTH = 4              # token passes (512 each)
THT = S // NTH       # 512
NKT = S // 128       # 16 key tiles of 128
SCALE = 1.0 / math.sqrt(HD)

_cached = None


def _build():
    nc = bacc.Bacc(None, target_bir_lowering=False)

    x = nc.dram_tensor("x", [S, D], BF16, kind="ExternalInput")
    wq = nc.dram_tensor("wq", [D, DH], BF16, kind="ExternalInput")
    wk = nc.dram_tensor("wk", [D, DH], BF16, kind="ExternalInput")
    wv = nc.dram_tensor("wv", [D, DH], BF16, kind="ExternalInput")
    wo = nc.dram_tensor("wo", [DH, D], BF16, kind="ExternalInput")
    cosb = nc.dram_tensor("cosb", [128, S], BF16, kind="ExternalInput")
    sinb = nc.dram_tensor("sinb", [128, S], BF16, kind="ExternalInput")
    pswap = nc.dram_tensor("pswap", [128, 128], BF16, kind="ExternalInput")
    outp = nc.dram_tensor("outp", [S, D], BF16, kind="ExternalOutput")

    with tile.TileContext(nc) as tc:
        with (
            tc.tile_pool(name="const", bufs=1) as cpool,
            tc.tile_pool(name="kq", bufs=1) as kqpool,
            tc.tile_pool(name="vaug", bufs=1) as vpool,
            tc.tile_pool(name="xt", bufs=3) as xtpool,
            tc.tile_pool(name="stream", bufs=2) as spool,
            tc.tile_pool(name="eab", bufs=1) as epool,
            tc.tile_pool(name="w0", bufs=1) as wpool,
        ):
            # resident weights; all big inputs go on the HWDGE (sync) path in
            # need-order (wk, x^T, trig chunk0, wq, trig rest, wv, wo); only
            # the tiny psw rides SWDGE so the Pool queue stays free early.
            wq_sb = wpool.tile([128, 8, DH], BF16, name="wq_sb")
            wk_sb = wpool.tile([128, 8, DH], BF16, name="wk_sb")
            wv_sb = wpool.tile([128, 8, DH], BF16, name="wv_sb")
            wo_sb = wpool.tile([128, 4, 2, DH], BF16, name="wo_sb")
            cos_t = cpool.tile([128, S], BF16, name="cos_t")
            sin_t = cpool.tile([128, S], BF16, name="sin_t")
            psw = cpool.tile([128, 128], BF16, name="psw")
            # PE p-state warmers: junk matmuls on memset tiles keep the PE
            # busy from ~0.4us so every real matmul is costed (and runs) at
            # the fully-ramped clock; the gate matmuls below park in the
            # 4-deep wait queue behind a DVE delay chain so the real matmuls
            # enter the pipeline only after the 3us ramp window has passed.
            jmm = cpool.tile([128, 512], BF16, name="jmm")
            dj0 = cpool.tile([128, 256], F32, name="dj0")
            dj1 = cpool.tile([128, 256], F32, name="dj1")
            ident = cpool.tile([128, 128], BF16, name="ident")
            nc.gpsimd.memset(jmm, 0.03125)
            nc.gpsimd.memset(dj0, 1.0)
            make_identity(nc, ident)
            # 0/1 causal masks for the final head's diagonal chunks, applied
            # as DVE multiplies (lower latency than Pool affine_select when
            # they gate the drain's AV chains)
            mtri = cpool.tile([128, 128], BF16, name="mtri")
            mtri2 = cpool.tile([128, 256], BF16, name="mtri2")
            nc.gpsimd.memset(mtri, 1.0)
            nc.gpsimd.memset(mtri2, 1.0)
            nc.gpsimd.affine_select(
                out=mtri, in_=mtri, compare_op=mybir.AluOpType.is_ge,
                fill=0.0, base=0, pattern=[[1, 128]], channel_multiplier=-1)
            nc.gpsimd.affine_select(
                out=mtri2, in_=mtri2, compare_op=mybir.AluOpType.is_ge,
                fill=0.0, base=-128, pattern=[[1, 256]], channel_multiplier=-1)

            # K^T / Q^T pair tiles: [128 dims (head 2p | head 2p+1), S]
            kt_tiles = [kqpool.tile([128, S], BF16, name=f"ktp{p}", tag=f"ktp{p}")
                        for p in range(NP)]
            qt_tiles = [kqpool.tile([128, S], BF16, name=f"qtp{p}", tag=f"qtp{p}")
                        for p in range(NP)]
            # V tiles with ones column: [128 keys, 8 heads, 64+1]
            v_tiles = [vpool.tile([128, 8, HD + 1], BF16, name=f"vt{t}", tag=f"vt{t}")
                       for t in range(NKT)]
            for t in range(NKT):
                nc.gpsimd.memset(v_tiles[t][:, :, HD], 1.0)

            # ctx^T accumulators, one per query block:
            # [128 dims-of-chunk, 4 chunks, 512 tokens]
            ctxT_qb = [kqpool.tile([128, 4, THT], BF16, name=f"ctxT{q}",
                                   tag=f"ctxT{q}") for q in range(NTH)]

            with (
                tc.tile_pool(name="pst", bufs=2, space="PSUM") as pst,
                tc.tile_pool(name="pssc", bufs=2, space="PSUM") as pssc,
                tc.tile_pool(name="psav", bufs=2, space="PSUM") as psav,
            ):
                # --- PE warm-up: primes + wait-queue gates (see above) ---
                jps = pst.tile([128, THT], F32, name="jps", tag="tp")
                for _ in range(4):
                    nc.tensor.matmul(jps, jmm[:, 0:128], jmm,
                                     start=True, stop=True)
                dsrc, ddst = dj0, dj1
                for _ in range(12):
                    nc.vector.tensor_copy(ddst, dsrc)
                    dsrc, ddst = ddst, dsrc
                for _ in range(4):
                    nc.tensor.matmul(jps[0:1, 0:1], dsrc[0:2, 0:1],
                                     dsrc[0:2, 0:1], start=True, stop=True)

                def oproj_nn(t, nn):
                    # output projection for token tile t, column half nn
                    cT = ctxT_qb[t // 4]
                    tl = t % 4
                    acc = pst.tile([128, DH], F32, name="oacc", tag="tp")
                    for c in range(4):
                        nc.tensor.matmul(
                            acc, cT[:, c, tl * 128:(tl + 1) * 128],
                            wo_sb[:, c, nn, :], start=(c == 0), stop=(c == 3))
                    osb = spool.tile([128, DH], BF16, name="osb", tag="osb",
                                     bufs=4)
                    nc.vector.tensor_copy(osb, acc)
                    nc.sync.dma_start(
                        out=outp[t * 128:(t + 1) * 128,
                                 nn * DH:(nn + 1) * DH],
                        in_=osb)

                def oproj(t):
                    oproj_nn(t, 0)
                    oproj_nn(t, 1)

                def do_xT(th):
                    # x^T for pass th via xbar DMA transpose:
                    # out[p, c, t] = x[th*512 + t, 128c + p]
                    xtb = xtpool.tile([128, 8, THT], BF16, name=f"xtb{th}", tag="xt")
                    nc.sync.dma_start_transpose(
                        out=xtb, in_=x[th * THT:(th + 1) * THT, :])
                    return xtb

                # pass-0 x^T: plain x load + PE tile-transposes instead of
                # the xbar DMA transpose — a transpose barriers the whole DMA
                # stream (each later DMA waits its completion semaphore), so
                # keeping it off the warmup path lets wk/trig stream behind x
                x_sb = xtpool.tile([128, 4, 1024], BF16, name="x_sb", tag="xt")
                for tb in range(4):
                    nc.sync.dma_start(out=x_sb[:, tb, :],
                                      in_=x[tb * 128:(tb + 1) * 128, :])
                wkr = wk.rearrange("(c p) j -> p c j", p=128)
                nc.sync.dma_start(out=wk_sb[:, 0:4, :], in_=wkr[:, 0:4, :])
                nc.sync.dma_start(out=wk_sb[:, 4:8, :], in_=wkr[:, 4:8, :])
                nc.sync.dma_start(out=cos_t[:, 0:THT], in_=cosb[:, 0:THT])
                nc.sync.dma_start(out=sin_t[:, 0:THT], in_=sinb[:, 0:THT])
                nc.sync.dma_start(out=psw, in_=pswap[:, :])
                nc.sync.dma_start(out=wq_sb,
                                  in_=wq.rearrange("(c p) j -> p c j", p=128))
                nc.sync.dma_start(out=wv_sb,
                                  in_=wv.rearrange("(c p) j -> p c j", p=128))
                nc.sync.dma_start(
                    out=wo_sb, in_=wo.rearrange("(c p) (n j) -> p c n j",
                                                p=128, n=2))
                for c_ in range(1, NTH):
                    nc.sync.dma_start(out=cos_t[:, c_ * THT:(c_ + 1) * THT],
                                      in_=cosb[:, c_ * THT:(c_ + 1) * THT])
                    nc.sync.dma_start(out=sin_t[:, c_ * THT:(c_ + 1) * THT],
                                      in_=sinb[:, c_ * THT:(c_ + 1) * THT])
                xtb = xtpool.tile([128, 8, THT], BF16, name="xtb0", tag="xt")
                for tb in range(4):
                    xtps = psav.tile([128, 8, 128], BF16, name="xtps",
                                     tag="av")
                    for c in range(8):
                        nc.tensor.transpose(
                            xtps[:, c, :],
                            x_sb[:, tb, c * 128:(c + 1) * 128], ident)
                    nc.vector.tensor_copy(
                        xtb[:, :, tb * 128:(tb + 1) * 128], xtps)
                pending_ctxT = []

                def rope_s1(wsb, dst, p, xtb_, t0_):
                    # projection chain + cos/sin products for one pair;
                    # returns the deferred swap+subtract stage
                    acc = pst.tile([128, THT], F32, name="acc", tag="tp")
                    for dc in range(8):
                        nc.tensor.matmul(
                            acc, wsb[:, dc, p * 128:(p + 1) * 128],
                            xtb_[:, dc, :], start=(dc == 0), stop=(dc == 7))
                    dslice = dst[p][:, t0_:t0_ + THT]
                    # sin product first: it gates the PE swap matmul
                    raw = spool.tile([128, THT], BF16, name="raw", tag="raw",
                                     bufs=4)
                    nc.vector.tensor_mul(raw, acc, sin_t[:, t0_:t0_ + THT])
                    nc.vector.tensor_mul(dslice, acc, cos_t[:, t0_:t0_ + THT])

                    def fin():
                        # r = a*cos - swap32(a*sin); the 32-row block swap
                        # runs on PE via a permutation matmul into PSUM
                        swp = psav.tile([128, THT], F32, name="swp", tag="av")
                        nc.tensor.matmul(swp, psw, raw, start=True, stop=True)
                        nc.vector.tensor_sub(dslice, dslice, swp)
                    return fin

                # pass-0 ropes, K-chains first: the four K projection chains
                # need only wk + x^T and stream back-to-back as soon as those
                # land; each swap stage and the Q chains slot in one step
                # behind as wq / trig tables arrive.
                p0_order = [(wk_sb, kt_tiles, p) for p in range(NP)] + \
                           [(wq_sb, qt_tiles, p) for p in range(NP)]
                p0_fin = None
                for wsb_, dst_, p_ in p0_order:
                    nxt = rope_s1(wsb_, dst_, p_, xtb, 0)
                    if p0_fin is not None:
                        p0_fin()
                    p0_fin = nxt
                p0_fin()

                for th in range(NTH):
                    t0 = th * THT
                    qb = th
                    q0 = qb * THT
                    nchunk = 2 * qb + 2

                    def do_rope(p, xtb_=None, t0_=None):
                        # Q^T/K^T projection + RoPE for head pair p; each
                        # swap stage lands after the sibling chain so PE
                        # never waits on the DVE cos/sin products
                        xtb_ = xtb if xtb_ is None else xtb_
                        t0_ = t0 if t0_ is None else t0_
                        fk = rope_s1(wk_sb, kt_tiles, p, xtb_, t0_)
                        fq = rope_s1(wq_sb, qt_tiles, p, xtb_, t0_)
                        fk()
                        fq()

                    def do_vproj_tl(tl):
                        acc = pst.tile([128, DH], F32, name="vacc", tag="tp")
                        for dc in range(8):
                            nc.tensor.matmul(
                                acc, xtb[:, dc, tl * 128:(tl + 1) * 128],
                                wv_sb[:, dc, :], start=(dc == 0), stop=(dc == 7))
                        vt = v_tiles[th * 4 + tl]
                        nc.vector.tensor_copy(
                            vt[:, :, 0:HD],
                            acc.rearrange("a (h d) -> a h d", h=8))

                    def do_vproj():
                        for tl in range(4):
                            do_vproj_tl(tl)

                    def do_scores(h, j, dve=False):
                        p, half = h // 2, h % 2
                        r0, r1 = 64 * half, 64 * half + 64
                        qoff = 256 if j == nchunk - 1 else 0
                        sc = pssc.tile([128, 2, THT], F32, name="sc", tag="sc")
                        for s_ in range(2):
                            kt = 2 * j + s_
                            # diag chunks: the second key tile's queries
                            # start 128 later; the unwritten PSUM strip is
                            # exp'd (stale scores stay small enough) and
                            # then zeroed by the mask fill
                            qs = qoff + (128 if s_ and j >= nchunk - 2 else 0)
                            nc.tensor.matmul(
                                sc[:, s_, qs:THT],
                                kt_tiles[p][r0:r1, kt * 128:(kt + 1) * 128],
                                qt_tiles[p][r0:r1, q0 + qs:q0 + THT],
                                start=True, stop=True)
                        eab = epool.tile([128, 2, THT], BF16, name="eab",
                                         tag="eab", bufs=26)
                        nc.scalar.activation(
                            eab[:, :, qoff:THT], sc[:, :, qoff:THT],
                            mybir.ActivationFunctionType.Exp, scale=SCALE)
                        if j >= nchunk - 2:
                            # diag masking; keys rel 0..255 (even chunk,
                            # cols 0..) or 256..511 (odd chunk, cols 256..).
                            # dve=True (final head) masks via 0/1-mask
                            # multiplies: lower latency than Pool when the
                            # drain AV chains are waiting
                            co = 0 if j == nchunk - 2 else 256
                            if dve:
                                nc.vector.tensor_mul(
                                    eab[:, 0, co:co + 128],
                                    eab[:, 0, co:co + 128], mtri)
                                nc.vector.tensor_mul(
                                    eab[:, 1, co:co + 256],
                                    eab[:, 1, co:co + 256], mtri2)
                            else:
                                nc.gpsimd.affine_select(
                                    out=eab[:, 0, co:co + 128],
                                    in_=eab[:, 0, co:co + 128],
                                    compare_op=mybir.AluOpType.is_ge, fill=0.0,
                                    base=0, pattern=[[1, 128]],
                                    channel_multiplier=-1)
                                nc.gpsimd.affine_select(
                                    out=eab[:, 1, co:co + 256],
                                    in_=eab[:, 1, co:co + 256],
                                    compare_op=mybir.AluOpType.is_ge, fill=0.0,
                                    base=-128, pattern=[[1, 256]],
                                    channel_multiplier=-1)
                        return eab

                    def do_scores_kt(h, j, s_, eab=None):
                        # single-kt scores+exp for the even-diagonal chunk of
                        # the final head, so its two key tiles can straddle
                        # the odd-diagonal chunk in the exp stream
                        p, half = h // 2, h % 2
                        r0, r1 = 64 * half, 64 * half + 64
                        kt = 2 * j + s_
                        sc = pssc.tile([128, 2, THT], F32, name="sc", tag="sc")
                        nc.tensor.matmul(
                            sc[:, s_, :],
                            kt_tiles[p][r0:r1, kt * 128:(kt + 1) * 128],
                            qt_tiles[p][r0:r1, q0:q0 + THT],
                            start=True, stop=True)
                        if eab is None:
                            eab = epool.tile([128, 2, THT], BF16, name="eab",
                                             tag="eab", bufs=26)
                        nc.scalar.activation(
                            eab[:, s_, :], sc[:, s_, :],
                            mybir.ActivationFunctionType.Exp, scale=SCALE)
                        if s_ == 0:
                            nc.gpsimd.affine_select(
                                out=eab[:, 0, 0:128], in_=eab[:, 0, 0:128],
                                compare_op=mybir.AluOpType.is_ge, fill=0.0,
                                base=0, pattern=[[1, 128]],
                                channel_multiplier=-1)
                        else:
                            nc.gpsimd.affine_select(
                                out=eab[:, 1, 0:256], in_=eab[:, 1, 0:256],
                                compare_op=mybir.AluOpType.is_ge, fill=0.0,
                                base=-128, pattern=[[1, 256]],
                                channel_multiplier=-1)
                        return eab

                    # normalized ctx staging: [128 queries, 512 dims] per qt
                    csts = [spool.tile([128, DH], BF16, name=f"cst{qt}",
                                       tag=f"cst{qt}", bufs=2) for qt in range(4)]

                    def do_av(h, qt, eabs_h):
                        # AV (flipped): out [128 queries, 65]
                        qt_g = 4 * qb + qt
                        av = psav.tile([128, 128], F32, name="av", tag="av")
                        for kt in range(qt_g + 1):
                            nc.tensor.matmul(
                                av[:, 0:HD + 1],
                                eabs_h[kt // 2][:, kt % 2,
                                                qt * 128:(qt + 1) * 128],
                                v_tiles[kt][:, h, :],
                                start=(kt == 0), stop=(kt == qt_g))
                        rec = spool.tile([128, 1], F32, name="rec", tag="rec",
                                         bufs=4)
                        nc.vector.reciprocal(rec, av[:, HD:HD + 1])
                        nc.vector.tensor_scalar(
                            out=csts[qt][:, HD * h:HD * (h + 1)],
                            in0=av[:, 0:HD], scalar1=rec, scalar2=None,
                            op0=mybir.AluOpType.mult)

                    # pass-3 split output projections for the final query
                    # block: ctx^T dim-chunk c holds heads 2c/2c+1 only, so
                    # chunks 0/1 transpose and project mid-pass; partials
                    # stage in a dead x^T buffer in bf16
                    p3h = {}

                    def tr_half(qt, hi):
                        # PE tile-transpose of csts[qt] dim-chunks 2hi,2hi+1;
                        # the hi=1 copies run in the drain where ACT is idle
                        # (and can read PSUM), keeping DVE free for the adds
                        tps = psav.tile([128, 2, 128], BF16, name="tps",
                                        tag="av")
                        for c in range(2):
                            cc = 2 * hi + c
                            nc.tensor.transpose(
                                tps[:, c, :],
                                csts[qt][:, cc * 128:(cc + 1) * 128], ident)
                        dst = ctxT_qb[3][:, 2 * hi:2 * hi + 2,
                                         qt * 128:(qt + 1) * 128]
                        if hi == 1:
                            nc.scalar.activation(
                                dst, tps, mybir.ActivationFunctionType.Copy)
                        else:
                            nc.vector.tensor_copy(dst, tps)

                    def _p3part():
                        if "part" not in p3h:
                            p3h["part"] = xtpool.tile([128, 8, THT], BF16,
                                                      name="part", tag="xt")
                        return p3h["part"]

                    def oproj_h1_nn(t, nn):
                        part = _p3part()
                        tl = t % 4
                        acc = pst.tile([128, DH], F32, name="oacc", tag="tp")
                        for c in range(2):
                            nc.tensor.matmul(
                                acc, ctxT_qb[3][:, c, tl * 128:(tl + 1) * 128],
                                wo_sb[:, c, nn, :], start=(c == 0),
                                stop=(c == 1))
                        nc.vector.tensor_copy(
                            part[:, (t - 12) * 2 + nn, :], acc)

                    def oproj_h2(t, split_last=False):
                        part = _p3part()
                        tl = t % 4
                        for nn in range(2):
                            # alternate acc pools: the score PSUM banks are
                            # free once the final exps drain, giving 4
                            # effective acc buffers in the tail
                            apool = pst if nn == 0 else pssc
                            acc = apool.tile([128, DH], F32, name="oacc",
                                             tag="tp" if nn == 0 else "sc")
                            for c in range(2, 4):
                                nc.tensor.matmul(
                                    acc, ctxT_qb[3][:, c, tl * 128:(tl + 1) * 128],
                                    wo_sb[:, c, nn, :], start=(c == 2),
                                    stop=(c == 3))
                            osb = spool.tile([128, DH], BF16, name="osb",
                                             tag="osb", bufs=4)
                            pslot = part[:, (t - 12) * 2 + nn, :]
                            nc.vector.tensor_add(osb, acc, pslot)
                            nc.sync.dma_start(
                                out=outp[t * 128:(t + 1) * 128,
                                         nn * DH:(nn + 1) * DH],
                                in_=osb)

                    # software-pipelined pass: rope pairs, V-proj, next-pass
                    # x^T and oproj are spread through the head stream so PE
                    # always has ready work while ACT drains exps
                    for fn in pending_ctxT:
                        fn()
                    pending_ctxT = []
                    eabs_prev = None
                    for h in range(8):
                        if th < 3:
                            # flat emission: scores burst, then prefetch
                            # work, then the previous head's AV chains
                            eabs = [do_scores(h, j) for j in range(nchunk)]
                            if h == 0:
                                do_vproj()
                                next_xtb = do_xT(th + 1)
                            elif 3 <= h <= 6:
                                do_rope(h - 3, xtb_=next_xtb,
                                        t0_=(th + 1) * THT)
                            if eabs_prev is not None:
                                for qt in range(4):
                                    do_av(h - 1, qt, eabs_prev)
                            eabs_prev = eabs
                            continue
                        # pass 3 is ACT-bound: weave the exp-independent
                        # work (vproj, deferred output projections, the
                        # final block's c=0/1 projection halves) between
                        # score chunks in ~1us granules so the exp stream
                        # never starves; AV chains (which wait on exps)
                        # stay after the score burst. The last head's
                        # diagonal chunk is emitted after its AV block so
                        # AV(7, 0/1), which don't need it, overlap its exp
                        last = h == 7
                        nj = nchunk - 1 if last else nchunk
                        eabs = [do_scores(h, j, dve=(last and j == nj - 1))
                                for j in range(nj)]
                        if h == 0:
                            do_vproj()
                        if eabs_prev is not None:
                            for qt in range(4):
                                do_av(h - 1, qt, eabs_prev)
                        # spread deferred output projections through the
                        # last pass; for the final query block, heads 0-3
                        # of ctx^T are complete once AV(3,*) lands, so the
                        # c=0/1 halves of its projections run at heads 5-6,
                        # leaving only the c=2/3 halves for the drain
                        oproj(h)
                        if h in (2, 3, 4):
                            oproj(8 + (2, 3, 4).index(h))
                        elif h == 5:
                            oproj(11)
                            for qt_ in range(4):
                                tr_half(qt_, 0)
                            for t_ in (12, 13):
                                oproj_h1_nn(t_, 0)
                                oproj_h1_nn(t_, 1)
                        elif h == 6:
                            for t_ in (14, 15):
                                oproj_h1_nn(t_, 0)
                                oproj_h1_nn(t_, 1)
                        if last:
                            eabs.append(do_scores(h, nchunk - 1, dve=True))
                        eabs_prev = eabs
                    if th < 3:
                        for qt in range(4):
                            do_av(7, qt, eabs_prev)
                        # ctx^T via xbar DMA: [128 q, 512 d] -> [128, 4, 128]
                        def mk(qb_, csts_):
                            def emit():
                                for qt in range(4):
                                    nc.sync.dma_start_transpose(
                                        out=ctxT_qb[qb_][:, :,
                                                         qt * 128:(qt + 1) * 128],
                                        in_=csts_[qt])
                            return emit
                        pending_ctxT.append(mk(qb, csts))
                        xtb = next_xtb
                    else:
                        # last pass drain: AV for qt 0/1 needs only chunks
                        # <=6, so it overlaps the final diagonal exp; only
                        # the c=2/3 projection halves plus the partial adds
                        # remain after the last AV chains
                        do_av(7, 0, eabs_prev)
                        tr_half(0, 1)
                        do_av(7, 1, eabs_prev)
                        tr_half(1, 1)
                        oproj_h2(12)
                        do_av(7, 2, eabs_prev)
                        tr_half(2, 1)
                        oproj_h2(13)
                        do_av(7, 3, eabs_prev)
                        tr_half(3, 1)
                        oproj_h2(14)
                        oproj_h2(15, split_last=True)

    nc.compile()
    return nc


def _host_tables(token_positions):
    pos = np.asarray(token_positions, dtype=np.float64)
    inv_freq = np.exp(np.arange(0, HD, 2, dtype=np.float64) * (-math.log(THETA) / HD))
    ang = pos[:, None] * inv_freq[None, :]  # [S, 32]
    cos = np.cos(ang).astype(np.float32).T  # [32, S]
    sin = np.sin(ang).astype(np.float32).T
    C = np.empty((128, S), np.float32)
    Sx = np.empty((128, S), np.float32)
    for half in range(2):
        r = 64 * half
        C[r:r + 32] = cos
        C[r + 32:r + 64] = cos
        Sx[r:r + 32] = -sin
        Sx[r + 32:r + 64] = sin
    return C, Sx


def kernel(in_features, token_positions, wq, wk, wv, wo):
    global _cached
    if _cached is None:
        _cached = _build()
    nc = _cached
    import ml_dtypes
    bf16 = ml_dtypes.bfloat16

    x = np.asarray(in_features, dtype=np.float32)
    perm = np.concatenate(
        [64 * h + np.concatenate([np.arange(0, 64, 2), np.arange(1, 64, 2)])
         for h in range(H)])
    wqp = np.ascontiguousarray(wq[:, perm]).astype(bf16)
    wkp = np.ascontiguousarray(wk[:, perm]).astype(bf16)
    wv = np.asarray(wv, dtype=np.float32).astype(bf16)
    wo = np.asarray(wo, dtype=np.float32).astype(bf16)
    C, Sx = _host_tables(token_positions)
    C = C.astype(bf16)
    Sx = Sx.astype(bf16)
    # 32-row block-swap permutation: out = P @ raw, P[i, swap(i)] = 1,
    # matmul computes lhsT.T @ rhs with lhsT = P^T, so store P^T = P (symmetric)
    P = np.zeros((128, 128), np.float32)
    for i in range(128):
        P[i ^ 32, i] = 1.0
    P = P.astype(bf16)

    in_maps = []
    for c in range(8):
        b, g = c // 2, c % 2
        sl = slice(g * DH, (g + 1) * DH)
        in_maps.append({
            "x": np.ascontiguousarray(x[b]).astype(bf16),
            "wq": np.ascontiguousarray(wqp[:, sl]),
            "wk": np.ascontiguousarray(wkp[:, sl]),
            "wv": np.ascontiguousarray(wv[:, sl]),
            "wo": np.ascontiguousarray(wo[sl, :]),
            "cosb": C,
            "sinb": Sx,
            "pswap": P,
        })
    results = _run(nc, in_maps)
    out = np.empty((B, S, D), np.float32)
    for b in range(B):
        out[b] = (results[2 * b]["outp"].astype(np.float32)
                  + results[2 * b + 1]["outp"].astype(np.float32))
    return out


_jit_cache = None


def _run(nc, in_maps):
    """Run the SPMD program on 8 cores, caching the jitted executable."""
    global _jit_cache
    try:
        import jax
        from jax.sharding import Mesh, PartitionSpec
        from jax.experimental.shard_map import shard_map
        from concourse import bass2jax
        import concourse.mybir as mybir_

        if _jit_cache is None:
            bass2jax.install_neuronx_cc_hook()
            pid_name = nc.partition_id_tensor.name if nc.partition_id_tensor else None
            in_names, out_names, out_avals, zero_outs = [], [], [], []
            for alloc in nc.m.functions[0].allocations:
                if not isinstance(alloc, mybir_.MemoryLocationSet):
                    continue
                nm = alloc.memorylocations[0].name
                if alloc.kind == "ExternalInput":
                    if nm != pid_name:
                        in_names.append(nm)
                elif alloc.kind == "ExternalOutput":
                    out_names.append(nm)
                    shape = tuple(alloc.tensor_shape)
                    dtype = mybir_.dt.np(alloc.dtype)
                    out_avals.append(jax.core.ShapedArray(shape, dtype))
                    zero_outs.append(np.zeros(shape, dtype))
            n_params = len(in_names)
            all_names = in_names + out_names
            if pid_name is not None:
                all_names = all_names + [pid_name]

            def _body(*args):
                operands = list(args)
                if pid_name is not None:
                    operands.append(bass2jax.partition_id_tensor())
                outs = bass2jax._bass_exec_p.bind(
                    *operands, out_avals=tuple(out_avals), in_names=tuple(all_names),
                    out_names=tuple(out_names), lowering_input_output_aliases=(),
                    sim_require_finite=True, sim_require_nnan=True, nc=nc)
                return tuple(outs)

            devices = jax.devices()[:8]
            mesh = Mesh(np.asarray(devices), ("core",))
            nio = n_params + len(out_names)
            sharded = jax.jit(
                shard_map(_body, mesh=mesh, in_specs=(PartitionSpec("core"),) * nio,
                          out_specs=(PartitionSpec("core"),) * len(out_names),
                          check_rep=False),
                keep_unused=True)
            _jit_cache = (sharded, in_names, out_names, zero_outs)

        sharded, in_names, out_names, zero_outs = _jit_cache
        concat_in = [np.concatenate([np.asarray(m[nm]) for m in in_maps], axis=0)
                     for nm in in_names]
        concat_zero = [np.concatenate([z] * 8, axis=0) for z in zero_outs]
        outs = sharded(*concat_in, *concat_zero)
        results = []
        for c in range(8):
            d = {}
            for i, nm in enumerate(out_names):
                arr = np.asarray(outs[i])
                n0 = arr.shape[0] // 8
                d[nm] = arr[c * n0:(c + 1) * n0]
            results.append(d)
        return results
    except Exception:
        res = run_bass_kernel_spmd(nc, in_maps, core_ids=list(range(8)))
        return res.results

